# revision 35
# baseline (speedup 1.0000x reference)
# MMoE Trainium2 Bass kernel.
#
# Reference computation (per batch row x of size 1024):
#   per expert e:  h = x@W1[e]+b1[e]; g1 = gelu(LN(h)*ln_g+ln_b); eo = gelu(g1@W2[e]+b2[e])
#   gates (3 tasks): gh = gelu([x,cemb]@Gw1+Gb1); w = softmax(gh@Gw2+Gb2)
#   out[t] = sum_e w[t,e] * eo[e]
#
# Strategy: data-parallel over batch across 8 cores (2048 rows each, processed
# in 2 halves of 1024).  All matmuls run in fp16 with fp32 PSUM accumulation
# (fp16 has 10 mantissa bits vs bf16's 7 at the same tensor-engine speed, and
# 2-byte dtypes double DVE elementwise throughput).
#
# Key structural points:
#  - LN mean subtraction is linear in x, so it is folded into the weights on
#    the host: W1' = W1 - rowmean(W1).  The matmul emits centered h directly;
#    no mean columns, no mean broadcast.  Variance is then just sum(h^2)/H.
#  - Expert layer 1 runs "transposed" (hidden on partitions, batch on free) so
#    LN gamma/beta fold into the Gelu activation as per-partition scale/bias;
#    layer 2 flips back to batch-on-partitions so softmax gate weights apply
#    as per-partition scalars.
#  - The per-(expert,col) work is software-pipelined one iteration deep so the
#    tensor engine's in-order queue never waits on the LN chain: PE order is
#    [l1(j) m0,m1][var-matmul(j-1)][l1(j) m2..7][l2(j-1)].
#  - Engine assignment: PSUM drains on Act (scalar.copy), squares/tree/rstd
#    muls on DVE (fp16 SBUF ops run at 2x), gate-weighted accumulation on the
#    otherwise-idle Pool/GpSimd engine, rstd partition-broadcast via a DRAM
#    bounce on the DVE DMA queue, input/weight loads on the SP queue, output
#    stores on the Pool queue.  This keeps any DMA semaphore wait from
#    blocking unrelated DMAs.
import numpy as np

_F16 = np.float16

B_FULL = 16384
IN_DIM = 1024
D_HID = 1024
D_EXP = 512
NE = 8
NT = 3
DC = 64
GH = 96  # 3 tasks x 32 gate hidden, concatenated
N_CORES = 8
EPS = 1e-5


def build_program(BC=2048, HALF=1024, has_b1=False, has_b2=False, has_gb2=False):
    import concourse.bass as bass
    import concourse.mybir as mybir
    from concourse import bacc
    from concourse.tile import TileContext

    dt = mybir.dt
    F32 = dt.float32
    F16 = dt.float16
    AF = mybir.ActivationFunctionType
    ALU = mybir.AluOpType

    NHALF = BC // HALF
    NBCOL = HALF // 512
    NBT = HALF // 128
    KI = IN_DIM // 128
    KH = D_HID // 128

    nc = bacc.Bacc("TRN2", target_bir_lowering=False)

    xt = nc.dram_tensor("xt", [IN_DIM, BC], F16, kind="ExternalInput")
    cta = nc.dram_tensor("cta", [DC + 1, BC], F16, kind="ExternalInput")
    w1f = nc.dram_tensor("w1f", [NE, IN_DIM + 1, D_HID], F16, kind="ExternalInput")
    w2a = nc.dram_tensor("w2a", [NE, D_HID + 1, D_EXP], F16, kind="ExternalInput")
    g1t = nc.dram_tensor("g1t", [IN_DIM, GH], F16, kind="ExternalInput")
    g1b = nc.dram_tensor("g1b", [DC + 1, GH], F16, kind="ExternalInput")
    g2bd = nc.dram_tensor("g2bd", [GH, NT * NE], F16, kind="ExternalInput")
    g2bias = nc.dram_tensor("g2bias", [1, NT * NE], F16, kind="ExternalInput")
    lng = nc.dram_tensor("lng", [128, NE * KH], F32, kind="ExternalInput")
    lnb = nc.dram_tensor("lnb", [128, NE * KH], F32, kind="ExternalInput")
    outs = [
        nc.dram_tensor(f"out{t}", [BC, D_EXP], F16, kind="ExternalOutput")
        for t in range(NT)
    ]

    with TileContext(nc) as tc:
        with (
            # fp16 partial sums are safe here: tree-added h^2 terms are
            # <= ~25 each, 1024 of them stays far below fp16 max (65504)
            # and per-add rounding is 2^-11
            nc.allow_low_precision(reason="fp16 variance tree / rstd"),
            tc.tile_pool(name="consts", bufs=1) as consts,
            tc.tile_pool(name="perhalf", bufs=1) as perhalf,
            tc.tile_pool(name="perhalf2", bufs=2) as perhalf2,
            tc.tile_pool(name="weights", bufs=2) as weights,
            tc.tile_pool(name="hcp", bufs=2) as hcp,
            tc.tile_pool(name="hsqp", bufs=2) as hsqp,
            tc.tile_pool(name="g1p", bufs=2) as g1p,
            tc.tile_pool(name="rsp", bufs=2) as rsp,
            tc.tile_pool(name="tmpp", bufs=3) as tmpp,
            tc.tile_pool(name="eop", bufs=3) as eop,
            tc.tile_pool(name="ph", bufs=4, space="PSUM") as ph_pool,
            tc.tile_pool(name="psq", bufs=2, space="PSUM") as psq_pool,
            tc.tile_pool(name="pz", bufs=2, space="PSUM") as pz_pool,
            tc.tile_pool(name="dscratch", bufs=2, space="DRAM") as dscratch,
        ):
            # ---- startup-critical loads first, in first-consumed order:
            # the k=0 slice of xt, the gate stationary, then the rest;
            # bulk constants afterwards (transfers serialize on the DMA
            # engines, so issue order is arrival order) ----
            halves = {}
            g1t_sb = consts.tile([128, KI, GH], F16, tag="g1t")

            def load_half(half, first=False):
                hs = slice(half * HALF, (half + 1) * HALF)
                xt_sb = perhalf2.tile([128, KI, HALF], F16, tag="xt")
                xr = xt[:, hs].rearrange("(k p) b -> p k b", p=128)
                if first:
                    nc.sync.dma_start(out=xt_sb[:, 0:1, :], in_=xr[:, 0:1, :])
                    nc.sync.dma_start(
                        out=g1t_sb,
                        in_=g1t[:, :].rearrange("(k p) m -> p k m", p=128),
                    )
                    nc.sync.dma_start(out=xt_sb[:, 1:2, :], in_=xr[:, 1:2, :])
                    for q in range(1, 4):
                        nc.sync.dma_start(
                            out=xt_sb[:, 2 * q : 2 * q + 2, :],
                            in_=xr[:, 2 * q : 2 * q + 2, :],
                        )
                else:
                    for q in range(4):
                        nc.sync.dma_start(
                            out=xt_sb[:, 2 * q : 2 * q + 2, :],
                            in_=xr[:, 2 * q : 2 * q + 2, :],
                        )
                cta_sb = perhalf2.tile([DC + 1, HALF], F16, tag="cta")
                nc.sync.dma_start(out=cta_sb, in_=cta[:, hs])
                halves[half] = dict(xt_sb=xt_sb, cta_sb=cta_sb)

            load_half(0, first=True)
            g1b_sb = consts.tile([DC + 1, GH], F16, tag="g1b")
            nc.sync.dma_start(out=g1b_sb, in_=g1b[:, :])
            g2bd_sb = consts.tile([GH, NT * NE], F16, tag="g2bd")
            nc.sync.dma_start(out=g2bd_sb, in_=g2bd[:, :])
            g2bias_sb = consts.tile([1, NT * NE], F16, tag="g2bias")
            nc.sync.dma_start(out=g2bias_sb, in_=g2bias[:, :])
            lng_sb = consts.tile([128, NE * KH], F32, tag="lng")
            nc.sync.dma_start(out=lng_sb, in_=lng[:, :])
            lnb_sb = consts.tile([128, NE * KH], F32, tag="lnb")
            nc.sync.dma_start(out=lnb_sb, in_=lnb[:, :])
            ones_row = consts.tile([1, HALF], F16, tag="ones_row")
            nc.vector.memset(ones_row, 1.0)
            ones_col = consts.tile([128, 1], F16, tag="ones_col")
            nc.vector.memset(ones_col, 1.0)
            eps_sb = consts.tile([1, 1], F32, tag="eps")
            nc.vector.memset(eps_sb, EPS)

            def gates_mm(half):
                st = halves[half]
                xt_sb, cta_sb = st["xt_sb"], st["cta_sb"]
                ghT_sb = perhalf.tile([GH, HALF], F16, tag="ghT")
                w_sb = perhalf2.tile([128, NBT, NT * NE], F32, tag="w")
                for c in range(NBCOL):
                    cs = slice(c * 512, (c + 1) * 512)
                    gh_ps = ph_pool.tile([GH, 512], F32, tag="ph")
                    for k in range(KI):
                        nc.tensor.matmul(
                            gh_ps,
                            g1t_sb[:, k, :],
                            xt_sb[:, k, cs],
                            start=(k == 0),
                            stop=False,
                        )
                    nc.tensor.matmul(
                        gh_ps, g1b_sb[:, :], cta_sb[:, cs], start=False, stop=True
                    )
                    nc.scalar.activation(ghT_sb[:, cs], gh_ps, AF.Gelu)
                    # logits for this col-chunk right away (they only read
                    # the freshly gelu'd ghT columns)
                    for bt in range(4 * c, 4 * c + 4):
                        bs = slice(bt * 128, (bt + 1) * 128)
                        lg_ps = pz_pool.tile([128, NT * NE], F32, tag="pz")
                        nc.tensor.matmul(
                            lg_ps,
                            ghT_sb[:, bs],
                            g2bd_sb[:, :],
                            start=True,
                            stop=not has_gb2,
                        )
                        if has_gb2:
                            nc.tensor.matmul(
                                lg_ps,
                                ones_row[0:1, 0:128],
                                g2bias_sb[:, :],
                                start=False,
                                stop=True,
                            )
                        nc.scalar.copy(w_sb[:, bt, :], lg_ps)
                st["w_sb"] = w_sb
                st["accs"] = [
                    perhalf.tile([128, NBT, D_EXP], F16, tag=f"acc{t}", name=f"acc{t}")
                    for t in range(NT)
                ]

            def gates_softmax(half):
                # deferred so its exp chain doesn't sit ahead of the first
                # expert's rstd chain in the Act queue; only needed by the
                # first stageC of the half
                st = halves[half]
                w_sb = st["w_sb"]
                nmx = perhalf2.tile([128, NBT * NT], F32, tag="nmx")
                ssum = perhalf2.tile([128, NBT * NT], F32, tag="ssum")
                rs = perhalf2.tile([128, NBT * NT], F32, tag="rs")
                nc.vector.tensor_reduce(
                    nmx[:, :],
                    w_sb[:].rearrange("p a (t e) -> p a t e", e=NE),
                    axis=mybir.AxisListType.X,
                    op=ALU.max,
                    negate=True,
                )
                for bt in range(NBT):
                    for t in range(NT):
                        j = bt * NT + t
                        nc.scalar.activation(
                            w_sb[:, bt, t * NE : (t + 1) * NE],
                            w_sb[:, bt, t * NE : (t + 1) * NE],
                            AF.Exp,
                            bias=nmx[:, j : j + 1],
                            accum_out=ssum[:, j : j + 1],
                        )
                nc.vector.reciprocal(rs[:, :], ssum[:, :])
                for bt in range(NBT):
                    for t in range(NT):
                        j = bt * NT + t
                        nc.vector.tensor_scalar_mul(
                            w_sb[:, bt, t * NE : (t + 1) * NE],
                            w_sb[:, bt, t * NE : (t + 1) * NE],
                            rs[:, j : j + 1],
                        )

            # flattened (half, expert, col) iteration list, pipelined one deep
            iters = [
                (h, e, c)
                for h in range(NHALF)
                for e in range(NE)
                for c in range(NBCOL)
            ]
            NITER = len(iters)
            state = [None] * NITER
            ew = {}  # expert -> (w1_sb, w2_sb) for the currently-loaded experts

            def load_expert(h, e):
                w1_sb = weights.tile([128, KI, D_HID], F16, tag="w1")
                w1r = w1f[e, 0:IN_DIM, :].rearrange("(k p) m -> p k m", p=128)
                for q in range(4):
                    nc.sync.dma_start(
                        out=w1_sb[:, 2 * q : 2 * q + 2, :],
                        in_=w1r[:, 2 * q : 2 * q + 2, :],
                    )
                w2_sb = weights.tile([128, KH, D_EXP], F16, tag="w2")
                w2r = w2a[e, 0:D_HID, :].rearrange("(k p) m -> p k m", p=128)
                nc.sync.dma_start(out=w2_sb[:, 0 : KH // 2, :], in_=w2r[:, 0 : KH // 2, :])
                nc.sync.dma_start(out=w2_sb[:, KH // 2 :, :], in_=w2r[:, KH // 2 :, :])
                ew[e] = (w1_sb, w2_sb)

            def l1_mtile(j, m):
                """One hidden m-tile of expert layer 1 + its drain and square."""
                h, e, c = iters[j]
                st = state[j]
                cs = st["cs"]
                xt_sb = halves[h]["xt_sb"]
                w1_sb = ew[e][0]
                hp = ph_pool.tile([128, 512], F32, tag="ph")
                for k in range(KI):
                    nc.tensor.matmul(
                        hp,
                        w1_sb[:, k, m * 128 : (m + 1) * 128],
                        xt_sb[:, k, cs],
                        start=(k == 0),
                        stop=(k == KI - 1),
                    )
                # drain PSUM -> fp16 SBUF on the scalar engine (Copy lives in
                # every act table, so no table swap)
                nc.scalar.copy(st["hc"][:, m, :], hp)
                nc.vector.tensor_mul(
                    st["hsq"][:, m, :], st["hc"][:, m, :], st["hc"][:, m, :]
                )
                # tree-add partials as soon as both inputs exist
                if m >= KH // 2:
                    lo = m - KH // 2
                    nc.vector.tensor_add(
                        st["hsq"][:, lo, :], st["hsq"][:, lo, :], st["hsq"][:, m, :]
                    )
                if m == KH - 1:
                    for step in (2, 1):
                        for i in range(step):
                            nc.vector.tensor_add(
                                st["hsq"][:, i, :],
                                st["hsq"][:, i, :],
                                st["hsq"][:, i + step, :],
                            )

            def stageA_open(j):
                h, e, c = iters[j]
                if e not in ew:
                    load_expert(h, e)  # only reached at j == 0
                hc = hcp.tile([128, KH, 512], F16, tag="hc", name="hc")
                hsq = hsqp.tile([128, KH, 512], F16, tag="hsq", name="hsq")
                st = state[j] = dict(
                    cs=slice(c * 512, (c + 1) * 512), hc=hc, hsq=hsq
                )
                l1_mtile(j, 0)
                l1_mtile(j, 1)

            def stageA_close(j):
                for m in range(2, KH):
                    l1_mtile(j, m)

            def stageB1_pe(j):
                """Variance matmul (partition-reduce of the tree-added h^2)."""
                st = state[j]
                sq_ps = psq_pool.tile([1, 512], F32, tag="psq")
                nc.tensor.matmul(
                    sq_ps, ones_col[:, 0:1], st["hsq"][:, 0, :], start=True, stop=True
                )
                st["sq_ps"] = sq_ps

            def stageB1_rest(j):
                """rstd chain + normalize muls for iter j."""
                h, e, c = iters[j]
                st = state[j]
                sq_ps = st["sq_ps"]
                rstd1 = rsp.tile([1, 512], F32, tag="rstd1")
                nc.scalar.activation(
                    rstd1, sq_ps, AF.Sqrt, bias=eps_sb[0:1, 0:1], scale=1.0 / D_HID
                )
                rstd = rsp.tile([1, 512], F16, tag="rstd")
                nc.vector.reciprocal(rstd, rstd1)
                rstd_d = dscratch.tile([1, 512], F16, tag="rstd_d")
                nc.scalar.dma_start(out=rstd_d, in_=rstd[0:1, :])
                rstd_b = rsp.tile([128, 512], F16, tag="rstd_b")
                nc.sync.dma_start(out=rstd_b, in_=rstd_d[:].to_broadcast([128, 512]))
                g1T = g1p.tile([128, KH, 512], F16, tag="g1T", name="g1T")
                st["g1T"] = g1T
                # normalize: hc * rstd.  hsq slots m>=1 are dead after the
                # tree-add (the variance matmul reads only slot 0, and rstd
                # already depends on it), so reuse them as outputs; slot 0
                # gets a scratch tile.
                tmp0 = tmpp.tile([128, 512], F16, tag="tmp")
                st["tmps"] = [tmp0] + [st["hsq"][:, m, :] for m in range(1, KH)]
                for m in range(KH):
                    nc.vector.tensor_mul(st["tmps"][m], st["hc"][:, m, :], rstd_b)

            def stageB2(j):
                h, e, c = iters[j]
                st = state[j]
                for m in range(KH):
                    nc.scalar.activation(
                        st["g1T"][:, m, :],
                        st["tmps"][m],
                        AF.Gelu,
                        bias=lnb_sb[:, e * KH + m : e * KH + m + 1],
                        scale=lng_sb[:, e * KH + m : e * KH + m + 1],
                    )

            def stageC(j):
                h, e, c = iters[j]
                st = state[j]
                hst = halves[h]
                w2_sb = ew[e][1]
                w_sb = hst["w_sb"]
                accs = hst["accs"]
                for mb in range(4):
                    bt = c * 4 + mb
                    bs = slice(mb * 128, (mb + 1) * 128)
                    z2 = pz_pool.tile([128, D_EXP], F32, tag="pz")
                    for k in range(KH):
                        nc.tensor.matmul(
                            z2,
                            st["g1T"][:, k, bs],
                            w2_sb[:, k, :],
                            start=(k == 0),
                            stop=(k == KH - 1),
                        )
                    eo = eop.tile([128, D_EXP], F16, tag="eo")
                    nc.scalar.activation(eo, z2, AF.Gelu)
                    for t in range(NT):
                        wsl = w_sb[:, bt, t * NE + e : t * NE + e + 1]
                        if e == 0:
                            nc.vector.tensor_scalar_mul(accs[t][:, bt, :], eo, wsl)
                        else:
                            nc.vector.scalar_tensor_tensor(
                                accs[t][:, bt, :],
                                eo,
                                wsl,
                                accs[t][:, bt, :],
                                op0=ALU.mult,
                                op1=ALU.add,
                            )
                    if e == NE - 1 and j == NITER - 1:
                        # final iteration: per-tile stores, but emitted after
                        # all eo gelus of this iteration so the Act queue
                        # never makes a gelu wait behind a store's
                        # accumulation semaphore
                        fs = st.setdefault("final_stores", [])
                        for t in range(NT):
                            rows = slice(
                                h * HALF + bt * 128, h * HALF + (bt + 1) * 128
                            )
                            fs.append((outs[t][rows, :], accs[t], bt))
                if e == NE - 1 and j != NITER - 1:
                    # one coarse store per task for this col-chunk (512 rows)
                    rows = slice(h * HALF + c * 512, h * HALF + (c + 1) * 512)
                    orr = [
                        outs[t][rows, :].rearrange("(a p) d -> p a d", p=128)
                        for t in range(NT)
                    ]
                    for t in range(NT):
                        nc.scalar.dma_start(
                            out=orr[t], in_=accs[t][:, c * 4 : (c + 1) * 4, :]
                        )
                for dst, acc_t, sbt in st.get("final_stores", []):
                    nc.scalar.dma_start(out=dst, in_=acc_t[:, sbt, :])
                if c == NBCOL - 1:
                    # expert fully consumed; let the weights pool recycle
                    del ew[e]

            gates_mm(0)
            pending_softmax = 0
            for j in range(NITER):
                h, e, c = iters[j]
                if h > 0 and (e, c) == (0, 0):
                    gates_mm(h)
                    pending_softmax = h
                stageA_open(j)
                if j >= 1:
                    stageB1_pe(j - 1)
                    stageB1_rest(j - 1)
                # prefetch the next iteration's expert weights now, AFTER the
                # rstd bounce DMAs so the weight transfers queue behind them
                if j + 1 < NITER:
                    nh, ne, ncol = iters[j + 1]
                    if ne not in ew:
                        load_expert(nh, ne)
                stageA_close(j)
                if j == NITER - 1:
                    # flush ordering: variance matmul for the last iteration
                    # right after its layer-1 (eats a short tree wait), the
                    # previous iteration's gelus next on the Act queue (so
                    # they don't sit behind the last sqrt), then the last
                    # rstd chain, all overlapping l2(j-1)
                    stageB1_pe(j)
                    stageB2(j - 1)
                    stageB1_rest(j)
                    stageC(j - 1)
                elif j >= 1:
                    stageB2(j - 1)
                    stageC(j - 1)
                if pending_softmax is not None:
                    gates_softmax(pending_softmax)
                    pending_softmax = None
                # prefetch the next half's inputs a few iterations early
                nh, ne, ncol = iters[min(j + 3, NITER - 1)]
                if nh not in halves:
                    load_half(nh)
            stageB2(NITER - 1)
            stageC(NITER - 1)

    nc.compile()
    return nc


def _host_prep(h_val, h_aro, cluster_id, W1, b1, ln_g, ln_b, W2, b2, emb, Gw1, Gb1, Gw2, Gb2):
    f32 = np.float32
    X = np.concatenate([h_val, h_aro], axis=1).astype(f32)
    B = X.shape[0]
    XT = np.ascontiguousarray(X.T).astype(_F16)
    cemb = np.asarray(emb, f32)[np.asarray(cluster_id).astype(np.int64)]
    cta = np.concatenate(
        [np.ascontiguousarray(cemb.T), np.ones((1, B), f32)], axis=0
    ).astype(_F16)

    W1 = np.asarray(W1, f32)
    b1 = np.asarray(b1, f32)
    W1a = np.concatenate([W1, b1[:, None, :]], axis=1)  # [E, 1025, 1024]
    # LN mean-subtraction is linear in x: fold it into the weights by
    # centering every row over the hidden dim
    W1c = W1a - W1a.mean(axis=2, keepdims=True, dtype=np.float64).astype(f32)
    w1f = W1c.astype(_F16)  # [E, 1025, 1024]

    W2 = np.asarray(W2, f32)
    b2 = np.asarray(b2, f32)
    w2a = np.concatenate([W2, b2[:, None, :]], axis=1).astype(_F16)  # [E, 1025, 512]

    Gw1 = np.asarray(Gw1, f32)  # [T, 1088, 32]
    Gb1 = np.asarray(Gb1, f32)  # [T, 32]
    G1 = np.concatenate([Gw1[t] for t in range(NT)], axis=1)  # [1088, 96]
    G1b_bias = np.concatenate([Gb1[t] for t in range(NT)], axis=0)[None, :]  # [1, 96]
    g1t = np.ascontiguousarray(G1[:IN_DIM]).astype(_F16)  # [1024, 96]
    g1b = np.concatenate([G1[IN_DIM:], G1b_bias], axis=0).astype(_F16)  # [65, 96]

    Gw2 = np.asarray(Gw2, f32)  # [T, 32, 8]
    Gb2 = np.asarray(Gb2, f32)  # [T, 8]
    g2bd = np.zeros((GH, NT * NE), f32)
    for t in range(NT):
        g2bd[t * 32 : (t + 1) * 32, t * NE : (t + 1) * NE] = Gw2[t]
    g2bd = g2bd.astype(_F16)
    g2bias = np.concatenate([Gb2[t] for t in range(NT)], axis=0)[None, :].astype(_F16)

    ln_g = np.asarray(ln_g, f32)
    ln_b = np.asarray(ln_b, f32)
    KH = D_HID // 128
    lng = np.ascontiguousarray(
        ln_g.reshape(NE, KH, 128).transpose(2, 0, 1).reshape(128, NE * KH)
    ).astype(f32)
    lnb = np.ascontiguousarray(
        ln_b.reshape(NE, KH, 128).transpose(2, 0, 1).reshape(128, NE * KH)
    ).astype(f32)

    shared = dict(
        w1f=w1f, w2a=w2a, g1t=g1t, g1b=g1b, g2bd=g2bd, g2bias=g2bias,
        lng=lng, lnb=lnb,
    )
    flags = dict(
        has_b1=bool(np.any(b1)), has_b2=bool(np.any(b2)), has_gb2=bool(np.any(Gb2)),
    )
    return XT, cta, shared, flags


def kernel_run(inputs, trace=False):
    import sys
    if "/opt/trn_rl_repo" not in sys.path:
        sys.path.insert(0, "/opt/trn_rl_repo")
    from concourse.bass_utils import run_bass_kernel_spmd

    XT, cta, shared, flags = _host_prep(**inputs)
    B = XT.shape[1]
    BC = B // N_CORES

    nc = build_program(BC=BC, HALF=1024, **flags)

    in_maps = []
    for c in range(N_CORES):
        cs = slice(c * BC, (c + 1) * BC)
        m = dict(shared)
        m["xt"] = np.ascontiguousarray(XT[:, cs])
        m["cta"] = np.ascontiguousarray(cta[:, cs])
        in_maps.append(m)

    res = run_bass_kernel_spmd(
        nc, in_maps, core_ids=list(range(N_CORES)), trace=trace
    )
    outs = []
    for t in range(NT):
        outs.append(
            np.concatenate(
                [res.results[c][f"out{t}"] for c in range(N_CORES)], axis=0
            ).astype(np.float32)
        )
    return tuple(outs), res


def kernel(h_val, h_aro, cluster_id, W1, b1, ln_g, ln_b, W2, b2, emb, Gw1, Gb1, Gw2, Gb2):
    outs, _ = kernel_run(
        dict(
            h_val=h_val, h_aro=h_aro, cluster_id=cluster_id, W1=W1, b1=b1,
            ln_g=ln_g, ln_b=ln_b, W2=W2, b2=b2, emb=emb,
            Gw1=Gw1, Gb1=Gb1, Gw2=Gw2, Gb2=Gb2,
        )
    )
    return outs


if __name__ == "__main__":
    print("kernel module loaded")


# revision 36
# speedup vs baseline: 1.0002x; 1.0002x over previous
# MMoE Trainium2 Bass kernel.
#
# Reference computation (per batch row x of size 1024):
#   per expert e:  h = x@W1[e]+b1[e]; g1 = gelu(LN(h)*ln_g+ln_b); eo = gelu(g1@W2[e]+b2[e])
#   gates (3 tasks): gh = gelu([x,cemb]@Gw1+Gb1); w = softmax(gh@Gw2+Gb2)
#   out[t] = sum_e w[t,e] * eo[e]
#
# Strategy: data-parallel over batch across 8 cores (2048 rows each, processed
# in 2 halves of 1024).  All matmuls run in fp16 with fp32 PSUM accumulation
# (fp16 has 10 mantissa bits vs bf16's 7 at the same tensor-engine speed, and
# 2-byte dtypes double DVE elementwise throughput).
#
# Key structural points:
#  - LN mean subtraction is linear in x, so it is folded into the weights on
#    the host: W1' = W1 - rowmean(W1).  The matmul emits centered h directly;
#    no mean columns, no mean broadcast.  Variance is then just sum(h^2)/H.
#  - Expert layer 1 runs "transposed" (hidden on partitions, batch on free) so
#    LN gamma/beta fold into the Gelu activation as per-partition scale/bias;
#    layer 2 flips back to batch-on-partitions so softmax gate weights apply
#    as per-partition scalars.
#  - The per-(expert,col) work is software-pipelined one iteration deep so the
#    tensor engine's in-order queue never waits on the LN chain: PE order is
#    [l1(j) m0,m1][var-matmul(j-1)][l1(j) m2..7][l2(j-1)].
#  - Engine assignment: PSUM drains on Act (scalar.copy), squares/tree/rstd
#    muls on DVE (fp16 SBUF ops run at 2x), gate-weighted accumulation on the
#    otherwise-idle Pool/GpSimd engine, rstd partition-broadcast via a DRAM
#    bounce on the DVE DMA queue, input/weight loads on the SP queue, output
#    stores on the Pool queue.  This keeps any DMA semaphore wait from
#    blocking unrelated DMAs.
import numpy as np

_F16 = np.float16

B_FULL = 16384
IN_DIM = 1024
D_HID = 1024
D_EXP = 512
NE = 8
NT = 3
DC = 64
GH = 96  # 3 tasks x 32 gate hidden, concatenated
N_CORES = 8
EPS = 1e-5


def build_program(BC=2048, HALF=1024, has_b1=False, has_b2=False, has_gb2=False):
    import concourse.bass as bass
    import concourse.mybir as mybir
    from concourse import bacc
    from concourse.tile import TileContext

    dt = mybir.dt
    F32 = dt.float32
    F16 = dt.float16
    AF = mybir.ActivationFunctionType
    ALU = mybir.AluOpType

    NHALF = BC // HALF
    NBCOL = HALF // 512
    NBT = HALF // 128
    KI = IN_DIM // 128
    KH = D_HID // 128

    nc = bacc.Bacc("TRN2", target_bir_lowering=False)

    xt = nc.dram_tensor("xt", [IN_DIM, BC], F16, kind="ExternalInput")
    cta = nc.dram_tensor("cta", [DC + 1, BC], F16, kind="ExternalInput")
    w1f = nc.dram_tensor("w1f", [NE, IN_DIM + 1, D_HID], F16, kind="ExternalInput")
    w2a = nc.dram_tensor("w2a", [NE, D_HID + 1, D_EXP], F16, kind="ExternalInput")
    g1t = nc.dram_tensor("g1t", [IN_DIM, GH], F16, kind="ExternalInput")
    g1b = nc.dram_tensor("g1b", [DC + 1, GH], F16, kind="ExternalInput")
    g2bd = nc.dram_tensor("g2bd", [GH, NT * NE], F16, kind="ExternalInput")
    g2bias = nc.dram_tensor("g2bias", [1, NT * NE], F16, kind="ExternalInput")
    lng = nc.dram_tensor("lng", [128, NE * KH], F32, kind="ExternalInput")
    lnb = nc.dram_tensor("lnb", [128, NE * KH], F32, kind="ExternalInput")
    outs = [
        nc.dram_tensor(f"out{t}", [BC, D_EXP], F16, kind="ExternalOutput")
        for t in range(NT)
    ]

    with TileContext(nc) as tc:
        with (
            # fp16 partial sums are safe here: tree-added h^2 terms are
            # <= ~25 each, 1024 of them stays far below fp16 max (65504)
            # and per-add rounding is 2^-11
            nc.allow_low_precision(reason="fp16 variance tree / rstd"),
            tc.tile_pool(name="consts", bufs=1) as consts,
            tc.tile_pool(name="perhalf", bufs=1) as perhalf,
            tc.tile_pool(name="perhalf2", bufs=2) as perhalf2,
            tc.tile_pool(name="weights", bufs=2) as weights,
            tc.tile_pool(name="hcp", bufs=2) as hcp,
            tc.tile_pool(name="hsqp", bufs=2) as hsqp,
            tc.tile_pool(name="g1p", bufs=2) as g1p,
            tc.tile_pool(name="rsp", bufs=2) as rsp,
            tc.tile_pool(name="tmpp", bufs=3) as tmpp,
            tc.tile_pool(name="eop", bufs=3) as eop,
            tc.tile_pool(name="ph", bufs=4, space="PSUM") as ph_pool,
            tc.tile_pool(name="psq", bufs=2, space="PSUM") as psq_pool,
            tc.tile_pool(name="pz", bufs=2, space="PSUM") as pz_pool,
            tc.tile_pool(name="dscratch", bufs=2, space="DRAM") as dscratch,
        ):
            # ---- startup-critical loads first, in first-consumed order:
            # the k=0 slice of xt, the gate stationary, then the rest;
            # bulk constants afterwards (transfers serialize on the DMA
            # engines, so issue order is arrival order) ----
            halves = {}
            g1t_sb = consts.tile([128, KI, GH], F16, tag="g1t")

            def load_half(half, first=False):
                hs = slice(half * HALF, (half + 1) * HALF)
                xt_sb = perhalf2.tile([128, KI, HALF], F16, tag="xt")
                xr = xt[:, hs].rearrange("(k p) b -> p k b", p=128)
                if first:
                    nc.sync.dma_start(
                        out=g1t_sb,
                        in_=g1t[:, :].rearrange("(k p) m -> p k m", p=128),
                    )
                for q in range(4):
                    nc.sync.dma_start(
                        out=xt_sb[:, 2 * q : 2 * q + 2, :],
                        in_=xr[:, 2 * q : 2 * q + 2, :],
                    )
                cta_sb = perhalf2.tile([DC + 1, HALF], F16, tag="cta")
                nc.sync.dma_start(out=cta_sb, in_=cta[:, hs])
                halves[half] = dict(xt_sb=xt_sb, cta_sb=cta_sb)

            load_half(0, first=True)
            g1b_sb = consts.tile([DC + 1, GH], F16, tag="g1b")
            nc.sync.dma_start(out=g1b_sb, in_=g1b[:, :])
            g2bd_sb = consts.tile([GH, NT * NE], F16, tag="g2bd")
            nc.sync.dma_start(out=g2bd_sb, in_=g2bd[:, :])
            g2bias_sb = consts.tile([1, NT * NE], F16, tag="g2bias")
            nc.sync.dma_start(out=g2bias_sb, in_=g2bias[:, :])
            lng_sb = consts.tile([128, NE * KH], F32, tag="lng")
            nc.sync.dma_start(out=lng_sb, in_=lng[:, :])
            lnb_sb = consts.tile([128, NE * KH], F32, tag="lnb")
            nc.sync.dma_start(out=lnb_sb, in_=lnb[:, :])
            ones_row = consts.tile([1, HALF], F16, tag="ones_row")
            nc.vector.memset(ones_row, 1.0)
            ones_col = consts.tile([128, 1], F16, tag="ones_col")
            nc.vector.memset(ones_col, 1.0)
            eps_sb = consts.tile([1, 1], F32, tag="eps")
            nc.vector.memset(eps_sb, EPS)

            def gates_mm(half):
                st = halves[half]
                xt_sb, cta_sb = st["xt_sb"], st["cta_sb"]
                ghT_sb = perhalf.tile([GH, HALF], F16, tag="ghT")
                w_sb = perhalf2.tile([128, NBT, NT * NE], F32, tag="w")
                for c in range(NBCOL):
                    cs = slice(c * 512, (c + 1) * 512)
                    gh_ps = ph_pool.tile([GH, 512], F32, tag="ph")
                    for k in range(KI):
                        nc.tensor.matmul(
                            gh_ps,
                            g1t_sb[:, k, :],
                            xt_sb[:, k, cs],
                            start=(k == 0),
                            stop=False,
                        )
                    nc.tensor.matmul(
                        gh_ps, g1b_sb[:, :], cta_sb[:, cs], start=False, stop=True
                    )
                    nc.scalar.activation(ghT_sb[:, cs], gh_ps, AF.Gelu)
                    # logits for this col-chunk right away (they only read
                    # the freshly gelu'd ghT columns)
                    for bt in range(4 * c, 4 * c + 4):
                        bs = slice(bt * 128, (bt + 1) * 128)
                        lg_ps = pz_pool.tile([128, NT * NE], F32, tag="pz")
                        nc.tensor.matmul(
                            lg_ps,
                            ghT_sb[:, bs],
                            g2bd_sb[:, :],
                            start=True,
                            stop=not has_gb2,
                        )
                        if has_gb2:
                            nc.tensor.matmul(
                                lg_ps,
                                ones_row[0:1, 0:128],
                                g2bias_sb[:, :],
                                start=False,
                                stop=True,
                            )
                        nc.scalar.copy(w_sb[:, bt, :], lg_ps)
                st["w_sb"] = w_sb
                st["accs"] = [
                    perhalf.tile([128, NBT, D_EXP], F16, tag=f"acc{t}", name=f"acc{t}")
                    for t in range(NT)
                ]

            def gates_softmax(half):
                # deferred so its exp chain doesn't sit ahead of the first
                # expert's rstd chain in the Act queue; only needed by the
                # first stageC of the half
                st = halves[half]
                w_sb = st["w_sb"]
                nmx = perhalf2.tile([128, NBT * NT], F32, tag="nmx")
                ssum = perhalf2.tile([128, NBT * NT], F32, tag="ssum")
                rs = perhalf2.tile([128, NBT * NT], F32, tag="rs")
                nc.vector.tensor_reduce(
                    nmx[:, :],
                    w_sb[:].rearrange("p a (t e) -> p a t e", e=NE),
                    axis=mybir.AxisListType.X,
                    op=ALU.max,
                    negate=True,
                )
                for bt in range(NBT):
                    for t in range(NT):
                        j = bt * NT + t
                        nc.scalar.activation(
                            w_sb[:, bt, t * NE : (t + 1) * NE],
                            w_sb[:, bt, t * NE : (t + 1) * NE],
                            AF.Exp,
                            bias=nmx[:, j : j + 1],
                            accum_out=ssum[:, j : j + 1],
                        )
                nc.vector.reciprocal(rs[:, :], ssum[:, :])
                for bt in range(NBT):
                    for t in range(NT):
                        j = bt * NT + t
                        nc.vector.tensor_scalar_mul(
                            w_sb[:, bt, t * NE : (t + 1) * NE],
                            w_sb[:, bt, t * NE : (t + 1) * NE],
                            rs[:, j : j + 1],
                        )

            # flattened (half, expert, col) iteration list, pipelined one deep
            iters = [
                (h, e, c)
                for h in range(NHALF)
                for e in range(NE)
                for c in range(NBCOL)
            ]
            NITER = len(iters)
            state = [None] * NITER
            ew = {}  # expert -> (w1_sb, w2_sb) for the currently-loaded experts

            def load_expert(h, e):
                w1_sb = weights.tile([128, KI, D_HID], F16, tag="w1")
                w1r = w1f[e, 0:IN_DIM, :].rearrange("(k p) m -> p k m", p=128)
                for q in range(4):
                    nc.sync.dma_start(
                        out=w1_sb[:, 2 * q : 2 * q + 2, :],
                        in_=w1r[:, 2 * q : 2 * q + 2, :],
                    )
                w2_sb = weights.tile([128, KH, D_EXP], F16, tag="w2")
                w2r = w2a[e, 0:D_HID, :].rearrange("(k p) m -> p k m", p=128)
                nc.sync.dma_start(out=w2_sb[:, 0 : KH // 2, :], in_=w2r[:, 0 : KH // 2, :])
                nc.sync.dma_start(out=w2_sb[:, KH // 2 :, :], in_=w2r[:, KH // 2 :, :])
                ew[e] = (w1_sb, w2_sb)

            def l1_mtile(j, m):
                """One hidden m-tile of expert layer 1 + its drain and square."""
                h, e, c = iters[j]
                st = state[j]
                cs = st["cs"]
                xt_sb = halves[h]["xt_sb"]
                w1_sb = ew[e][0]
                hp = ph_pool.tile([128, 512], F32, tag="ph")
                for k in range(KI):
                    nc.tensor.matmul(
                        hp,
                        w1_sb[:, k, m * 128 : (m + 1) * 128],
                        xt_sb[:, k, cs],
                        start=(k == 0),
                        stop=(k == KI - 1),
                    )
                # drain PSUM -> fp16 SBUF on the scalar engine (Copy lives in
                # every act table, so no table swap)
                nc.scalar.copy(st["hc"][:, m, :], hp)
                nc.vector.tensor_mul(
                    st["hsq"][:, m, :], st["hc"][:, m, :], st["hc"][:, m, :]
                )
                # tree-add partials as soon as both inputs exist
                if m >= KH // 2:
                    lo = m - KH // 2
                    nc.vector.tensor_add(
                        st["hsq"][:, lo, :], st["hsq"][:, lo, :], st["hsq"][:, m, :]
                    )
                if m == KH - 1:
                    for step in (2, 1):
                        for i in range(step):
                            nc.vector.tensor_add(
                                st["hsq"][:, i, :],
                                st["hsq"][:, i, :],
                                st["hsq"][:, i + step, :],
                            )

            def stageA_open(j):
                h, e, c = iters[j]
                if e not in ew:
                    load_expert(h, e)  # only reached at j == 0
                hc = hcp.tile([128, KH, 512], F16, tag="hc", name="hc")
                hsq = hsqp.tile([128, KH, 512], F16, tag="hsq", name="hsq")
                st = state[j] = dict(
                    cs=slice(c * 512, (c + 1) * 512), hc=hc, hsq=hsq
                )
                l1_mtile(j, 0)
                l1_mtile(j, 1)

            def stageA_close(j):
                for m in range(2, KH):
                    l1_mtile(j, m)

            def stageB1_pe(j):
                """Variance matmul (partition-reduce of the tree-added h^2)."""
                st = state[j]
                sq_ps = psq_pool.tile([1, 512], F32, tag="psq")
                nc.tensor.matmul(
                    sq_ps, ones_col[:, 0:1], st["hsq"][:, 0, :], start=True, stop=True
                )
                st["sq_ps"] = sq_ps

            def stageB1_rest(j):
                """rstd chain + normalize muls for iter j."""
                h, e, c = iters[j]
                st = state[j]
                sq_ps = st["sq_ps"]
                rstd1 = rsp.tile([1, 512], F32, tag="rstd1")
                nc.scalar.activation(
                    rstd1, sq_ps, AF.Sqrt, bias=eps_sb[0:1, 0:1], scale=1.0 / D_HID
                )
                rstd = rsp.tile([1, 512], F16, tag="rstd")
                nc.vector.reciprocal(rstd, rstd1)
                rstd_d = dscratch.tile([1, 512], F16, tag="rstd_d")
                nc.scalar.dma_start(out=rstd_d, in_=rstd[0:1, :])
                rstd_b = rsp.tile([128, 512], F16, tag="rstd_b")
                nc.sync.dma_start(out=rstd_b, in_=rstd_d[:].to_broadcast([128, 512]))
                g1T = g1p.tile([128, KH, 512], F16, tag="g1T", name="g1T")
                st["g1T"] = g1T
                # normalize: hc * rstd.  hsq slots m>=1 are dead after the
                # tree-add (the variance matmul reads only slot 0, and rstd
                # already depends on it), so reuse them as outputs; slot 0
                # gets a scratch tile.
                tmp0 = tmpp.tile([128, 512], F16, tag="tmp")
                st["tmps"] = [tmp0] + [st["hsq"][:, m, :] for m in range(1, KH)]
                for m in range(KH):
                    nc.vector.tensor_mul(st["tmps"][m], st["hc"][:, m, :], rstd_b)

            def stageB2(j):
                h, e, c = iters[j]
                st = state[j]
                for m in range(KH):
                    nc.scalar.activation(
                        st["g1T"][:, m, :],
                        st["tmps"][m],
                        AF.Gelu,
                        bias=lnb_sb[:, e * KH + m : e * KH + m + 1],
                        scale=lng_sb[:, e * KH + m : e * KH + m + 1],
                    )

            def stageC(j):
                h, e, c = iters[j]
                st = state[j]
                hst = halves[h]
                w2_sb = ew[e][1]
                w_sb = hst["w_sb"]
                accs = hst["accs"]
                for mb in range(4):
                    bt = c * 4 + mb
                    bs = slice(mb * 128, (mb + 1) * 128)
                    z2 = pz_pool.tile([128, D_EXP], F32, tag="pz")
                    for k in range(KH):
                        nc.tensor.matmul(
                            z2,
                            st["g1T"][:, k, bs],
                            w2_sb[:, k, :],
                            start=(k == 0),
                            stop=(k == KH - 1),
                        )
                    eo = eop.tile([128, D_EXP], F16, tag="eo")
                    nc.scalar.activation(eo, z2, AF.Gelu)
                    for t in range(NT):
                        wsl = w_sb[:, bt, t * NE + e : t * NE + e + 1]
                        if e == 0:
                            nc.vector.tensor_scalar_mul(accs[t][:, bt, :], eo, wsl)
                        else:
                            nc.vector.scalar_tensor_tensor(
                                accs[t][:, bt, :],
                                eo,
                                wsl,
                                accs[t][:, bt, :],
                                op0=ALU.mult,
                                op1=ALU.add,
                            )
                    if e == NE - 1 and j == NITER - 1:
                        # final iteration: per-tile stores, but emitted after
                        # all eo gelus of this iteration so the Act queue
                        # never makes a gelu wait behind a store's
                        # accumulation semaphore
                        fs = st.setdefault("final_stores", [])
                        for t in range(NT):
                            rows = slice(
                                h * HALF + bt * 128, h * HALF + (bt + 1) * 128
                            )
                            fs.append((outs[t][rows, :], accs[t], bt))
                if e == NE - 1 and j != NITER - 1:
                    # one coarse store per task for this col-chunk (512 rows)
                    rows = slice(h * HALF + c * 512, h * HALF + (c + 1) * 512)
                    orr = [
                        outs[t][rows, :].rearrange("(a p) d -> p a d", p=128)
                        for t in range(NT)
                    ]
                    for t in range(NT):
                        nc.scalar.dma_start(
                            out=orr[t], in_=accs[t][:, c * 4 : (c + 1) * 4, :]
                        )
                for dst, acc_t, sbt in st.get("final_stores", []):
                    nc.scalar.dma_start(out=dst, in_=acc_t[:, sbt, :])
                if c == NBCOL - 1:
                    # expert fully consumed; let the weights pool recycle
                    del ew[e]

            gates_mm(0)
            pending_softmax = 0
            for j in range(NITER):
                h, e, c = iters[j]
                if h > 0 and (e, c) == (0, 0):
                    gates_mm(h)
                    pending_softmax = h
                stageA_open(j)
                if j >= 1:
                    stageB1_pe(j - 1)
                    stageB1_rest(j - 1)
                # prefetch the next iteration's expert weights now, AFTER the
                # rstd bounce DMAs so the weight transfers queue behind them
                if j + 1 < NITER:
                    nh, ne, ncol = iters[j + 1]
                    if ne not in ew:
                        load_expert(nh, ne)
                stageA_close(j)
                if j == NITER - 1:
                    # flush ordering: variance matmul for the last iteration
                    # right after its layer-1 (eats a short tree wait), the
                    # previous iteration's gelus next on the Act queue (so
                    # they don't sit behind the last sqrt), then the last
                    # rstd chain, all overlapping l2(j-1)
                    stageB1_pe(j)
                    stageB2(j - 1)
                    stageB1_rest(j)
                    stageC(j - 1)
                elif j >= 1:
                    stageB2(j - 1)
                    stageC(j - 1)
                if pending_softmax is not None:
                    gates_softmax(pending_softmax)
                    pending_softmax = None
                # prefetch the next half's inputs a few iterations early
                nh, ne, ncol = iters[min(j + 3, NITER - 1)]
                if nh not in halves:
                    load_half(nh)
            stageB2(NITER - 1)
            stageC(NITER - 1)

    nc.compile()
    return nc


def _host_prep(h_val, h_aro, cluster_id, W1, b1, ln_g, ln_b, W2, b2, emb, Gw1, Gb1, Gw2, Gb2):
    f32 = np.float32
    X = np.concatenate([h_val, h_aro], axis=1).astype(f32)
    B = X.shape[0]
    XT = np.ascontiguousarray(X.T).astype(_F16)
    cemb = np.asarray(emb, f32)[np.asarray(cluster_id).astype(np.int64)]
    cta = np.concatenate(
        [np.ascontiguousarray(cemb.T), np.ones((1, B), f32)], axis=0
    ).astype(_F16)

    W1 = np.asarray(W1, f32)
    b1 = np.asarray(b1, f32)
    W1a = np.concatenate([W1, b1[:, None, :]], axis=1)  # [E, 1025, 1024]
    # LN mean-subtraction is linear in x: fold it into the weights by
    # centering every row over the hidden dim
    W1c = W1a - W1a.mean(axis=2, keepdims=True, dtype=np.float64).astype(f32)
    w1f = W1c.astype(_F16)  # [E, 1025, 1024]

    W2 = np.asarray(W2, f32)
    b2 = np.asarray(b2, f32)
    w2a = np.concatenate([W2, b2[:, None, :]], axis=1).astype(_F16)  # [E, 1025, 512]

    Gw1 = np.asarray(Gw1, f32)  # [T, 1088, 32]
    Gb1 = np.asarray(Gb1, f32)  # [T, 32]
    G1 = np.concatenate([Gw1[t] for t in range(NT)], axis=1)  # [1088, 96]
    G1b_bias = np.concatenate([Gb1[t] for t in range(NT)], axis=0)[None, :]  # [1, 96]
    g1t = np.ascontiguousarray(G1[:IN_DIM]).astype(_F16)  # [1024, 96]
    g1b = np.concatenate([G1[IN_DIM:], G1b_bias], axis=0).astype(_F16)  # [65, 96]

    Gw2 = np.asarray(Gw2, f32)  # [T, 32, 8]
    Gb2 = np.asarray(Gb2, f32)  # [T, 8]
    g2bd = np.zeros((GH, NT * NE), f32)
    for t in range(NT):
        g2bd[t * 32 : (t + 1) * 32, t * NE : (t + 1) * NE] = Gw2[t]
    g2bd = g2bd.astype(_F16)
    g2bias = np.concatenate([Gb2[t] for t in range(NT)], axis=0)[None, :].astype(_F16)

    ln_g = np.asarray(ln_g, f32)
    ln_b = np.asarray(ln_b, f32)
    KH = D_HID // 128
    lng = np.ascontiguousarray(
        ln_g.reshape(NE, KH, 128).transpose(2, 0, 1).reshape(128, NE * KH)
    ).astype(f32)
    lnb = np.ascontiguousarray(
        ln_b.reshape(NE, KH, 128).transpose(2, 0, 1).reshape(128, NE * KH)
    ).astype(f32)

    shared = dict(
        w1f=w1f, w2a=w2a, g1t=g1t, g1b=g1b, g2bd=g2bd, g2bias=g2bias,
        lng=lng, lnb=lnb,
    )
    flags = dict(
        has_b1=bool(np.any(b1)), has_b2=bool(np.any(b2)), has_gb2=bool(np.any(Gb2)),
    )
    return XT, cta, shared, flags


def kernel_run(inputs, trace=False):
    import sys
    if "/opt/trn_rl_repo" not in sys.path:
        sys.path.insert(0, "/opt/trn_rl_repo")
    from concourse.bass_utils import run_bass_kernel_spmd

    XT, cta, shared, flags = _host_prep(**inputs)
    B = XT.shape[1]
    BC = B // N_CORES

    nc = build_program(BC=BC, HALF=1024, **flags)

    in_maps = []
    for c in range(N_CORES):
        cs = slice(c * BC, (c + 1) * BC)
        m = dict(shared)
        m["xt"] = np.ascontiguousarray(XT[:, cs])
        m["cta"] = np.ascontiguousarray(cta[:, cs])
        in_maps.append(m)

    res = run_bass_kernel_spmd(
        nc, in_maps, core_ids=list(range(N_CORES)), trace=trace
    )
    outs = []
    for t in range(NT):
        outs.append(
            np.concatenate(
                [res.results[c][f"out{t}"] for c in range(N_CORES)], axis=0
            ).astype(np.float32)
        )
    return tuple(outs), res


def kernel(h_val, h_aro, cluster_id, W1, b1, ln_g, ln_b, W2, b2, emb, Gw1, Gb1, Gw2, Gb2):
    outs, _ = kernel_run(
        dict(
            h_val=h_val, h_aro=h_aro, cluster_id=cluster_id, W1=W1, b1=b1,
            ln_g=ln_g, ln_b=ln_b, W2=W2, b2=b2, emb=emb,
            Gw1=Gw1, Gb1=Gb1, Gw2=Gw2, Gb2=Gb2,
        )
    )
    return outs


if __name__ == "__main__":
    print("kernel module loaded")
